# revision 4
# baseline (speedup 1.0000x reference)
"""DFA-GNN (max-aggregation message passing) Trainium2 kernel.

Problem (B=2, N=4096, E=65536, M=4, H=256), per batch b:
    coeff[e]  = edge_fts[b,e,:] @ edge_W + edge_b                  # [E]
    agg[n]    = max over edges e with tgt[e]==n of coeff[e] * hint[b, src[e]]
    out[b,n]  = (node_fts[b,n] + agg[n]) @ update_W + update_b     # [M,H] rows

Sharding: 8 cores = 2 batches x 4 target-node quarters (1024 nodes each).
Edges are bucketed by target node on the host (every node has exactly 16
incoming edges with this generator; general counts <=16 are padded by
duplicating an edge, which preserves the max).

v2 data path: hint rows are quantized to INT8 on the host with a per-node
row scale s[n] = max|row|/127 (rel err vs f32 reference ~8e-3, gate 2e-2).
The scale of the gathered source row folds into the per-edge coefficient:
t_e = (coeff_e + edge_b) * s[src_e], applied on device as one tiny [128,16]
STT per block. node_fts are also int8 (per-node scale), dequantized inside
the fused STT that adds them to the aggregated max. This halves the
dominant HBM traffic (gather 32->16.8MB/core, nf 2->1MB/core).

DMA is HBM-bandwidth-bound at ~22.3GB/s/engine x 16 = 358GB/s/core:
  gather 16.8MB + eftsT 8.4MB (bf16) + nf 1MB + out 2.1MB + idx 0.26MB
  ~= 28.6MB -> ~80us floor.

Per block (128 nodes, K=16 edge ranks):
  - 4x SWDGE dma_gather (512 descriptors of 1KB, round-robin 4 queues),
  - per-edge coeff via PE matmuls (eW stationary, host-transposed bf16
    edge features streaming) sprayed PSUM->[128,16] via DMA reshape,
  - t = (coeff + eb) * svec in one DVE STT,
  - mult+max chain on two ping-pong accumulator chains (out-of-place ops
    keep DVE 2x mode): ACT ranks {1,2,4,5,7,8,10,11,13,14,15} multiply on
    Act (int8 in, bf16 out, per-partition f32 scale) + DVE TT-max; STT
    ranks {0,3,6,9,12} run fused mult+max STT on DVE (1x; int8 input
    disables 2x anyway),
  - +node_fts via fused STT (int8 nf * s_nf + comb),
  - transpose to feature-major via DMA xbar transpose (sync HWDGE queue;
    frees PE of 8 transposes/block and Act of the xt copy),
  - update_W matmuls (8/block); edge_b always folded into t; update_b
    rank-1 matmuls only compiled when update_b != 0 host-side,
  - bf16 output, upcast to f32 on the host.

Things measured NOT to work on this hardware, kept so future sessions
skip them: fp8_e4m3 hint quantization fails accuracy (3.0e-2 > 2e-2);
fp8_e3m4 edge features pass only marginally (1.8e-2) - kept bf16;
multi-index indirect DMA corrupts data; ap_gather is ~9x its cost model;
Pool/GpSimd cannot run tensor_tensor / scalar_tensor_tensor (walrus
engine check) and bulk elementwise on it is software-slow;
int8 operands disable DVE 2x/4x perf modes (1-byte dtype breaks the
2-byte requirement), so int8 multiplies cost 731ns on Act and DVE alike;
in-place DVE/Act elementwise ops lose their perf mode (~15% penalty);
bf16 PSUM cannot accumulate on TRN2.
"""

import os
import sys

import numpy as np

for _p in ("/opt/trn_rl_repo", "/root/.axon_site/_ro/trn_rl_repo"):
    if os.path.isdir(_p) and _p not in sys.path:
        sys.path.insert(0, _p)

B, N, E, M, H = 2, 4096, 65536, 4, 256
MH = M * H            # 1024
P = 128               # partitions
K = 16                # edges per node (E // N)
NCORE = N // 4        # nodes per core (1024)
NB = NCORE // P       # node blocks per core (8)
EC = NCORE * K        # edges per core (16384)
ECB = P * K           # edges per block (2048)
KH = K // 4           # edge ranks per gather tile (4)
GT_BUFS = int(os.environ.get("KERNEL_GT_BUFS", "8"))
N_CORES = 8
N_SWDGE_Q = int(os.environ.get("KERNEL_SWDGE_Q", "4"))

# ranks whose multiply runs as a fused mult+max STT on DVE; the rest
# multiply on Act and max on DVE (rank 1 writes its chain head directly).
STT_RANKS = (0, 3, 6, 9, 12)

_CACHE = {}

# Set by kernel() when KERNEL_TRACE=1: BassKernelResults of the last run.
LAST_RESULT = None


def _build(with_ub: bool):
    from concourse import bass, bacc, mybir, tile

    f32 = mybir.dt.float32
    i16 = mybir.dt.int16
    i8 = mybir.dt.int8
    bf16 = mybir.dt.bfloat16

    nc = bacc.Bacc("TRN2", target_bir_lowering=False, debug=False,
                   num_devices=N_CORES, num_swdge_queues=N_SWDGE_Q)

    hint = nc.dram_tensor("hint", [N, MH], i8, kind="ExternalInput")
    eftsT = nc.dram_tensor("eftsT", [H, EC], bf16, kind="ExternalInput")
    idx_d = nc.dram_tensor("idx16", [P, EC // 16], i16, kind="ExternalInput")
    nf_d = nc.dram_tensor("nf", [NCORE, MH], i8, kind="ExternalInput")
    eW_d = nc.dram_tensor("eW", [P, 2], bf16, kind="ExternalInput")
    eb_d = nc.dram_tensor("eb", [P, 1], f32, kind="ExternalInput")
    uW_d = nc.dram_tensor("uW", [H, H], bf16, kind="ExternalInput")
    ub_d = nc.dram_tensor("ub", [1, H], bf16, kind="ExternalInput")
    sv_d = nc.dram_tensor("sv", [P, NB * K], f32, kind="ExternalInput")
    sn_d = nc.dram_tensor("sn", [P, NB], f32, kind="ExternalInput")
    out_d = nc.dram_tensor("out", [NCORE, MH], bf16, kind="ExternalOutput")

    with tile.TileContext(nc) as tc:
        from concourse.mybir import AluOpType as alu

        with (
            tc.tile_pool(name="const", bufs=1) as cpool,
            tc.tile_pool(name="efts", bufs=2) as epool,
            tc.tile_pool(name="gt", bufs=GT_BUFS) as gpool,
            tc.tile_pool(name="sc", bufs=4) as scpool,
            tc.tile_pool(name="work", bufs=2) as wpool,
            tc.tile_pool(name="ps_coeff", bufs=2, space="PSUM") as ps_coeff,
            tc.tile_pool(name="ps_out", bufs=2, space="PSUM") as ps_out,
        ):
            # idx16 rides the Act-engine HWDGE queue: the sync queue floods
            # with edge features at startup and would delay the first
            # gather's index table.
            idx_t = cpool.tile([P, EC // 16], i16)
            nc.scalar.dma_start(out=idx_t[:], in_=idx_d[:])
            eW = cpool.tile([P, 2], bf16)
            nc.scalar.dma_start(out=eW[:], in_=eW_d[:])
            eb = cpool.tile([P, 1], f32)
            nc.scalar.dma_start(out=eb[:], in_=eb_d[:])
            uW0 = cpool.tile([P, H], bf16)
            uW1 = cpool.tile([P, H], bf16)
            nc.scalar.dma_start(out=uW0[:], in_=uW_d[0:P, :])
            nc.scalar.dma_start(out=uW1[:], in_=uW_d[P:2 * P, :])
            sv_t = cpool.tile([P, NB * K], f32)
            nc.scalar.dma_start(out=sv_t[:], in_=sv_d[:])
            sn_t = cpool.tile([P, NB], f32)
            nc.scalar.dma_start(out=sn_t[:], in_=sn_d[:])
            if with_ub:
                ub_row = cpool.tile([1, H], bf16)
                nc.scalar.dma_start(out=ub_row[:], in_=ub_d[:])
                ones1 = cpool.tile([1, P], bf16)
                nc.vector.memset(ones1[:], 1.0)

            coeffs = [None] * NB

            def emit_coeff(nb):
                # eftsT columns node-major in the block:
                # col nb*2048 + p*16 + k -> edge rank k of node p.
                efts0 = epool.tile([P, ECB], bf16, tag="efts0")
                efts1 = epool.tile([P, ECB], bf16, tag="efts1")
                nc.sync.dma_start(out=efts0[:],
                                  in_=eftsT[0:P, nb * ECB:(nb + 1) * ECB])
                nc.sync.dma_start(out=efts1[:],
                                  in_=eftsT[P:2 * P, nb * ECB:(nb + 1) * ECB])
                co_ps = ps_coeff.tile([128, 1024], f32, tag="co_ps",
                                      space="PSUM")
                for c in range(4):
                    pp, ff = (c % 2) * 64, (c // 2) * 512
                    nc.tensor.matmul(co_ps[pp:pp + 1, ff:ff + 512],
                                     lhsT=eW[:, 0:1],
                                     rhs=efts0[:, c * 512:(c + 1) * 512],
                                     start=True, stop=False)
                    nc.tensor.matmul(co_ps[pp:pp + 1, ff:ff + 512],
                                     lhsT=eW[:, 1:2],
                                     rhs=efts1[:, c * 512:(c + 1) * 512],
                                     start=False, stop=True)
                # PSUM -> SBUF rows (DMA cannot read PSUM), then spray
                # [1,512] coeff rows into [32,16] per-partition layout.
                co_row = wpool.tile([P, 512], f32, tag="co_row")
                for c in range(4):
                    pp, ff = (c % 2) * 64, (c // 2) * 512
                    nc.scalar.copy(co_row[c * 32:c * 32 + 1, :],
                                   co_ps[pp:pp + 1, ff:ff + 512])
                coeff = wpool.tile([P, K], f32, tag="coeff")
                for c in range(4):
                    nc.sync.dma_start(
                        out=coeff[c * 32:(c + 1) * 32, :],
                        in_=co_row[c * 32:c * 32 + 1, :].rearrange(
                            "c (p k) -> c p k", k=K))
                coeffs[nb] = coeff

            emit_coeff(0)
            # one shared num_idxs register for every dma_gather (all 512)
            nidx_reg = nc.gpsimd.to_reg(P * KH)
            for nb in range(NB):
                # ---- gather: one SWDGE dma_gather per 4 edge ranks ----
                gts = []
                for h in range(K // KH):
                    gt = gpool.tile([P, KH * MH], i8, tag="gt")
                    c0 = (nb * ECB + h * P * KH) // 16
                    nc.gpsimd.dma_gather(
                        gt[:].rearrange("p (g e) -> p g e", e=MH),
                        hint[:],
                        idx_t[:, c0:c0 + P * KH // 16],
                        P * KH, nidx_reg, MH,
                        queue_num=(nb * (K // KH) + h) % N_SWDGE_Q,
                    )
                    gts.append(gt)
                nf = wpool.tile([P, MH], i8, tag="nf")
                nc.scalar.dma_start(out=nf[:], in_=nf_d[nb * P:(nb + 1) * P, :])
                if nb + 1 < NB:
                    emit_coeff(nb + 1)

                # t = (coeff + eb) * s_hint[src], all per-edge [128, K]
                t = wpool.tile([P, K], f32, tag="t")
                nc.vector.scalar_tensor_tensor(
                    out=t[:], in0=coeffs[nb][:], scalar=eb[:, 0:1],
                    in1=sv_t[:, nb * K:(nb + 1) * K],
                    op0=alu.add, op1=alu.mult)

                # ---- mult+max, two ping-pong chains ----
                # chain A: Act multiplies (rank 1 writes the head), DVE maxes
                # chain B: DVE tensor_scalar head (rank 0) + fused STTs
                accA = [wpool.tile([P, MH], bf16, tag=f"accA{i}",
                                   name=f"accA{i}") for i in range(2)]
                accB = [wpool.tile([P, MH], bf16, tag=f"accB{i}",
                                   name=f"accB{i}") for i in range(2)]
                ia = ib = 0
                na = nb_ = 0
                for h in range(K // KH):
                    gt = gts[h]
                    for j in range(KH):
                        k = h * KH + j
                        src = gt[:, j * MH:(j + 1) * MH]
                        sc = t[:, k:k + 1]
                        if k in STT_RANKS:
                            if nb_ == 0:
                                nc.vector.tensor_scalar(
                                    out=accB[0][:], in0=src, scalar1=sc,
                                    scalar2=None, op0=alu.mult)
                            else:
                                nc.vector.scalar_tensor_tensor(
                                    out=accB[1 - ib][:], in0=src, scalar=sc,
                                    in1=accB[ib][:], op0=alu.mult,
                                    op1=alu.max)
                                ib = 1 - ib
                            nb_ += 1
                        else:
                            if na == 0:
                                nc.scalar.mul(accA[0][:], src, sc)
                            else:
                                sct = scpool.tile([P, MH], bf16, tag="sct")
                                nc.scalar.mul(sct[:], src, sc)
                                nc.vector.tensor_tensor(
                                    out=accA[1 - ia][:], in0=sct[:],
                                    in1=accA[ia][:], op=alu.max)
                                ia = 1 - ia
                            na += 1

                # ---- combine, +node_fts (dequant fused), transpose ----
                comb = wpool.tile([P, MH], bf16, tag="comb")
                nc.vector.tensor_tensor(out=comb[:], in0=accA[ia][:],
                                        in1=accB[ib][:], op=alu.max)
                xf = wpool.tile([P, MH], bf16, tag="xf")
                nc.vector.scalar_tensor_tensor(
                    out=xf[:], in0=nf[:], scalar=sn_t[:, nb:nb + 1],
                    in1=comb[:], op0=alu.mult, op1=alu.add)
                xt = wpool.tile([P, MH], bf16, tag="xt")
                nc.sync.dma_start_transpose(
                    xt[:].rearrange("p (c n) -> p c n", n=P), xf[:])

                # ---- update matmuls ----
                o_ps = ps_out.tile([P, MH], f32, tag="o_ps", space="PSUM")
                for m in range(M):
                    nc.tensor.matmul(o_ps[:, m * H:(m + 1) * H],
                                     lhsT=xt[:, (2 * m) * P:(2 * m + 1) * P],
                                     rhs=uW0[:], start=True, stop=False)
                    nc.tensor.matmul(o_ps[:, m * H:(m + 1) * H],
                                     lhsT=xt[:, (2 * m + 1) * P:(2 * m + 2) * P],
                                     rhs=uW1[:], start=False,
                                     stop=not with_ub)
                    if with_ub:
                        nc.tensor.matmul(o_ps[:, m * H:(m + 1) * H],
                                         lhsT=ones1[0:1, :], rhs=ub_row[0:1, :],
                                         start=False, stop=True)
                o = wpool.tile([P, MH], bf16, tag="o")
                nc.scalar.copy(o[:], o_ps[:])
                nc.scalar.dma_start(out=out_d[nb * P:(nb + 1) * P, :], in_=o[:])

    nc.compile()
    return nc


def _install_ntff_hook():
    """Register the axon NTFF profiling hook if this image's antenv lacks it.

    Mirrors what trn_boot does when ``antenv.axon_hooks`` exists. Safe no-op
    on failure — tracing is skipped, execution still works.
    """
    import types

    try:
        import antenv.axon_hooks  # noqa: F401
        return
    except ImportError:
        pass
    try:
        import antenv
        from trn_agent_boot.trn_boot import _ntff_profile_via_ctypes

        hook = _ntff_profile_via_ctypes("/opt/axon/libaxon_pjrt.so")
        mod = types.ModuleType("antenv.axon_hooks")
        state = {"hook": hook}
        mod.get_axon_ntff_profile_hook = lambda: state["hook"]
        mod.set_axon_ntff_profile_hook = lambda h: state.update(hook=h)
        sys.modules["antenv.axon_hooks"] = mod
        antenv.axon_hooks = mod
    except Exception as e:  # pragma: no cover - best effort
        print(f"ntff hook install failed: {e}", file=sys.stderr)


def _edge_grid(tgt_b):
    """[N, K] edge ids bucketed by target node, padded by duplication."""
    counts = np.bincount(tgt_b, minlength=N)
    if counts.max() > K or counts.min() < 1:
        raise ValueError(f"edge counts per node outside [1, {K}]: "
                         f"min={counts.min()} max={counts.max()}")
    order = np.argsort(tgt_b, kind="stable")
    if (counts == K).all():
        return order.reshape(N, K)
    pos = np.zeros(N + 1, np.int64)
    np.cumsum(counts, out=pos[1:])
    offs = np.minimum(np.arange(K)[None, :], (counts - 1)[:, None])
    return order[pos[:-1, None] + offs]


def _quant_rows(x):
    """Symmetric int8 per-row quantization: x ~= q * s[:, None]."""
    a = np.abs(x).max(axis=1)
    s = np.where(a > 0, a, 1.0).astype(np.float32) / 127.0
    q = np.clip(np.rint(x / s[:, None]), -127, 127).astype(np.int8)
    return q, s


def kernel(**inputs):
    global LAST_RESULT
    import ml_dtypes
    from concourse.bass_utils import run_bass_kernel_spmd

    wdt = ml_dtypes.bfloat16

    cfg = np.asarray(inputs["cfg_indices_padded"])
    hint_state = np.asarray(inputs["hint_state"], dtype=np.float32)
    node_fts = np.asarray(inputs["node_fts"], dtype=np.float32)
    edge_fts = np.asarray(inputs["edge_fts"], dtype=np.float32)
    edge_W = np.asarray(inputs["edge_W"], dtype=np.float32)
    edge_b = np.asarray(inputs["edge_b"], dtype=np.float32)
    update_W = np.asarray(inputs["update_W"], dtype=np.float32)
    update_b = np.asarray(inputs["update_b"], dtype=np.float32)

    src = np.asarray(cfg[..., 0], dtype=np.int64)
    tgt = np.asarray(cfg[..., 1], dtype=np.int64)

    with_ub = bool(np.any(update_b != 0.0))
    key = ("nc", with_ub)
    if key not in _CACHE:
        _CACHE[key] = _build(with_ub)
    nc = _CACHE[key]

    eW_in = np.ascontiguousarray(edge_W[:, 0].reshape(2, P).T).astype(wdt)
    eb_in = np.full((P, 1), edge_b[0], np.float32)
    ub_in = np.ascontiguousarray(update_b[None, :]).astype(wdt)
    uW_in = update_W.astype(wdt)

    in_maps = []
    for b in range(B):
        hq, hs = _quant_rows(hint_state[b].reshape(N, MH))
        grid = _edge_grid(tgt[b])             # [N, K]
        srcg = src[b][grid]                   # [N, K]
        for q in range(4):
            g_q = grid[q * NCORE:(q + 1) * NCORE]    # [1024, K]
            s_q = srcg[q * NCORE:(q + 1) * NCORE]
            # gather index order: i = nb*2048 + k*128 + p, wrapped into
            # [16, EC/16] (idx16[r, c] = position c*16+r), tiled to 128 rows.
            gorder = s_q.reshape(NB, P, K).transpose(0, 2, 1)   # [nb, k, p]
            idx16 = np.ascontiguousarray(
                np.tile(gorder.reshape(EC // 16, 16).T, (8, 1))
            ).astype(np.int16)
            # per-edge hint-row scales in the coeff layout [p, nb*K + k]
            sv = np.ascontiguousarray(
                hs[s_q.reshape(NB, P, K)].transpose(1, 0, 2).reshape(
                    P, NB * K)).astype(np.float32)
            # edge-feature column order: j = nb*2048 + p*16 + k (node-major)
            eids = g_q.reshape(NB * P * K)
            efts_t = np.ascontiguousarray(edge_fts[b][eids].T).astype(wdt)
            nq, ns = _quant_rows(
                node_fts[b, q * NCORE:(q + 1) * NCORE].reshape(NCORE, MH))
            sn = np.ascontiguousarray(
                ns.reshape(NB, P).T).astype(np.float32)    # [p, nb]
            in_maps.append({
                "hint": hq,
                "eftsT": efts_t,
                "idx16": idx16,
                "nf": nq,
                "eW": eW_in,
                "eb": eb_in,
                "uW": uW_in,
                "ub": ub_in,
                "sv": sv,
                "sn": sn,
            })

    trace = bool(int(os.environ.get("KERNEL_TRACE", "0")))
    if trace:
        _install_ntff_hook()
    res = run_bass_kernel_spmd(nc, in_maps, core_ids=list(range(N_CORES)),
                               trace=trace)
    if trace:
        LAST_RESULT = res

    out = np.empty((B, N, M, H), np.float32)
    for b in range(B):
        for q in range(4):
            o = np.asarray(res.results[b * 4 + q]["out"], dtype=np.float32)
            out[b, q * NCORE:(q + 1) * NCORE] = o.reshape(NCORE, M, H)
    return out
